# revision 1
# baseline (speedup 1.0000x reference)
"""Trainium2 Bass kernel: segment-mean over token segments + pairwise-diff edge MLP.

Reference computation (per batch row b):
  seg = cumsum(ids == 3); valid = ids != 3
  means[n] = mean of features[s] over tokens with seg==n & valid (n < 8), 0-count -> sum/1
  diff[i,j] = means[i] - means[j]                          # [8,8,H]
  out[i,j]  = relu(relu(diff @ W1 + b1) @ Wm + bm) @ W2 + b2   # [8,8,150]

Distribution: data-parallel over batch B=128 across 8 NeuronCores (16 rows/core),
tiny MLP weights replicated, no cross-core communication.

Device algorithm per core:
  stage1: means^T-ish  [8seg, 768] per row via TensorE: onehot (stationary, 0/1,
          host-precomputed) x features (moving) accumulated over 8 token chunks,
          scaled by 1/count on PSUM->SBUF eviction (ScalarE activation scale).
  diff:   one matmul per (group-of-4-rows, h-chunk): diffT = means^T @ E4 where E4
          is a constant +-1 selection matrix -> fuses the transpose AND the
          pairwise difference. Output columns = (g2, b2, i, j) = 256 per 4 rows.
  MLP:    transposed matmuls, contraction dim on partitions, c-dim split 128+22.
          Biases b1/bm applied as per-partition activation bias (c on partitions);
          b2 added via a K=1 matmul with a ones row. Final out is [rows, 150].
"""

import sys

import numpy as np

if "/opt/trn_rl_repo" not in sys.path:
    sys.path.insert(0, "/opt/trn_rl_repo")

import concourse.bass as bass
import concourse.mybir as mybir
from concourse import bacc
from concourse.bass import ds
from concourse.bass_utils import run_bass_kernel_spmd
from concourse.tile import TileContext

B, S, H, C = 128, 1024, 768, 150
NSEG = 8
SEP_ID = 3
NCORES = 8
RPC = B // NCORES  # 16 rows per core
TCH = S // 128     # 8 token chunks
HC = H // 128      # 6 hidden chunks
HHALF = 384        # H split for PSUM bank limit
CC = ((0, 128), (128, 22))  # c-dim (150) chunks
CPAD = 256         # final free dim padded so fp32r runs full-rate

F32 = mybir.dt.float32
F32R = mybir.dt.float32r

# fp32r = single-pass fp32 matmul mode (reduced internal precision, 4x faster
# moving-dim throughput when free dim >= 256). Flags allow fp32 fallback.
F32R_STAGE1 = True
F32R_MLP = True


def build_program(rpc=RPC, tch=TCH, f32r_stage1=F32R_STAGE1, f32r_mlp=F32R_MLP,
                  feat_bufs=4):
    S_ = tch * 128
    ngp = rpc // 4  # group-pairs: 4 batch rows -> 256 output rows each
    nc = bass.Bass("TRN2", target_bir_lowering=False, debug=False)

    DT1 = F32R if f32r_stage1 else F32   # stage-1 matmul operand dtype
    DTM = F32R if f32r_mlp else F32      # MLP matmul operand dtype
    feats_d = nc.dram_tensor("features", [rpc, S_, H], DT1, kind="ExternalInput").ap()
    ohT_d = nc.dram_tensor("ohT", [128, rpc * tch * NSEG], DT1, kind="ExternalInput").ap()
    icnt_d = nc.dram_tensor("icnt", [NSEG, rpc], F32, kind="ExternalInput").ap()
    w1p_d = nc.dram_tensor("w1p", [128, HC * C], DTM, kind="ExternalInput").ap()
    wm0_d = nc.dram_tensor("wm0", [128, C], DTM, kind="ExternalInput").ap()
    wm1_d = nc.dram_tensor("wm1", [22, C], DTM, kind="ExternalInput").ap()
    w20_d = nc.dram_tensor("w20", [128, CPAD], DTM, kind="ExternalInput").ap()
    w21_d = nc.dram_tensor("w21", [22, CPAD], DTM, kind="ExternalInput").ap()
    b1c0_d = nc.dram_tensor("b1c0", [128, 1], F32, kind="ExternalInput").ap()
    b1c1_d = nc.dram_tensor("b1c1", [22, 1], F32, kind="ExternalInput").ap()
    bm0_d = nc.dram_tensor("bm0", [128, 1], F32, kind="ExternalInput").ap()
    bm1_d = nc.dram_tensor("bm1", [22, 1], F32, kind="ExternalInput").ap()
    b2p_d = nc.dram_tensor("b2pad", [1, CPAD], DTM, kind="ExternalInput").ap()
    e4_d = nc.dram_tensor("e4", [NSEG, 4 * 256], DTM, kind="ExternalInput").ap()
    ones_d = nc.dram_tensor("ones", [1, 128], DTM, kind="ExternalInput").ap()
    out_d = nc.dram_tensor("out", [ngp * 256, C], F32, kind="ExternalOutput").ap()

    RELU = mybir.ActivationFunctionType.Relu
    COPY = mybir.ActivationFunctionType.Copy

    with TileContext(nc) as tc:
        with (
            tc.tile_pool(name="const", bufs=1) as constp,
            tc.tile_pool(name="featp", bufs=feat_bufs) as featp,
            tc.tile_pool(name="meansp", bufs=8) as meansp,
            tc.tile_pool(name="diffp", bufs=2) as diffp,
            tc.tile_pool(name="actp", bufs=2) as actp,
            tc.tile_pool(name="osbp", bufs=3) as osbp,
            tc.tile_pool(name="mpsum", bufs=2, space="PSUM") as mpsum,
            tc.tile_pool(name="dpsum", bufs=2, space="PSUM") as dpsum,
            tc.tile_pool(name="hpsum", bufs=2, space="PSUM") as hpsum,
            tc.tile_pool(name="opsum", bufs=2, space="PSUM") as opsum,
        ):
            ohT_sb = constp.tile([128, rpc * tch * NSEG], DT1, tag="c_ohT")
            nc.gpsimd.dma_start(out=ohT_sb, in_=ohT_d)
            icnt_sb = constp.tile([NSEG, rpc], F32, tag="c_icnt")
            nc.gpsimd.dma_start(out=icnt_sb, in_=icnt_d)
            w1_sb = constp.tile([128, HC * C], DTM, tag="c_w1")
            nc.gpsimd.dma_start(out=w1_sb, in_=w1p_d)
            wm0_sb = constp.tile([128, C], DTM, tag="c_wm0")
            nc.gpsimd.dma_start(out=wm0_sb, in_=wm0_d)
            wm1_sb = constp.tile([22, C], DTM, tag="c_wm1")
            nc.gpsimd.dma_start(out=wm1_sb, in_=wm1_d)
            w20_sb = constp.tile([128, CPAD], DTM, tag="c_w20")
            nc.gpsimd.dma_start(out=w20_sb, in_=w20_d)
            w21_sb = constp.tile([22, CPAD], DTM, tag="c_w21")
            nc.gpsimd.dma_start(out=w21_sb, in_=w21_d)
            b1_sb = []
            for ci, (coff, csz) in enumerate(CC):
                t = constp.tile([csz, 1], F32, tag=f"c_b1_{ci}")
                nc.gpsimd.dma_start(out=t, in_=(b1c0_d, b1c1_d)[ci])
                b1_sb.append(t)
            bm_sb = []
            for ci, (coff, csz) in enumerate(CC):
                t = constp.tile([csz, 1], F32, tag=f"c_bm_{ci}")
                nc.gpsimd.dma_start(out=t, in_=(bm0_d, bm1_d)[ci])
                bm_sb.append(t)
            b2p_sb = constp.tile([1, CPAD], DTM, tag="c_b2")
            nc.gpsimd.dma_start(out=b2p_sb, in_=b2p_d)
            e4_sb = constp.tile([NSEG, 4 * 256], DTM, tag="c_e4")
            nc.gpsimd.dma_start(out=e4_sb, in_=e4_d)
            ones_sb = constp.tile([1, 128], DTM, tag="c_ones")
            nc.gpsimd.dma_start(out=ones_sb, in_=ones_d)

            for gp in range(ngp):
                # ---- stage 1: segment means for 4 batch rows ----
                means = []
                for r4 in range(4):
                    row = gp * 4 + r4
                    feat = featp.tile([128, tch, H], DT1, tag="feat")
                    dma_eng = nc.sync if (row % 2 == 0) else nc.scalar
                    dma_eng.dma_start(
                        out=feat,
                        in_=feats_d[row].rearrange("(t p) h -> p t h", p=128),
                    )
                    m = meansp.tile([NSEG, H], DTM, tag="means")
                    for half in range(2):
                        mp = mpsum.tile([NSEG, HHALF], F32, tag="mp")
                        for t in range(tch):
                            nc.tensor.matmul(
                                mp,
                                ohT_sb[:, ds(row * tch * NSEG + t * NSEG, NSEG)],
                                feat[:, t, ds(half * HHALF, HHALF)],
                                start=(t == 0),
                                stop=(t == tch - 1),
                            )
                        nc.scalar.activation(
                            m[:, ds(half * HHALF, HHALF)], mp, COPY,
                            scale=icnt_sb[:, ds(row, 1)],
                        )
                    means.append(m)

                # ---- pairwise diff (fused transpose): diffT = means^T @ E4 ----
                diff = diffp.tile([128, HC, 256], DTM, tag="diff")
                for hc in range(HC):
                    dp = dpsum.tile([128, 256], F32, tag="dp")
                    for r4 in range(4):
                        nc.tensor.matmul(
                            dp,
                            means[r4][:, ds(hc * 128, 128)],
                            e4_sb[:, ds(r4 * 256, 256)],
                            start=(r4 == 0),
                            stop=(r4 == 3),
                        )
                    nc.vector.tensor_copy(diff[:, hc, :], dp)

                # ---- mm1: h1T = relu(W1^T @ diffT + b1) ----
                h1 = []
                for ci, (coff, csz) in enumerate(CC):
                    hp = hpsum.tile([csz, 256], F32, tag="hp")
                    for hc in range(HC):
                        nc.tensor.matmul(
                            hp,
                            w1_sb[:, ds(hc * C + coff, csz)],
                            diff[:, hc, :],
                            start=(hc == 0),
                            stop=(hc == HC - 1),
                        )
                    hs = actp.tile([csz, 256], DTM, tag=f"h1s{ci}")
                    nc.scalar.activation(hs, hp, RELU, bias=b1_sb[ci])
                    h1.append(hs)

                # ---- mm2: h2T = relu(Wm^T @ h1T + bm) ----
                h2 = []
                for ci, (coff, csz) in enumerate(CC):
                    hp = hpsum.tile([csz, 256], F32, tag="hp")
                    nc.tensor.matmul(hp, wm0_sb[:, ds(coff, csz)],
                                     h1[0], start=True, stop=False)
                    nc.tensor.matmul(hp, wm1_sb[:, ds(coff, csz)],
                                     h1[1], start=False, stop=True)
                    hs = actp.tile([csz, 256], DTM, tag=f"h2s{ci}")
                    nc.scalar.activation(hs, hp, RELU, bias=bm_sb[ci])
                    h2.append(hs)

                # ---- mm3: out = h2 @ W2 + b2, natural [rows, c] layout ----
                for rs in range(2):
                    op = opsum.tile([128, CPAD], F32, tag="op")
                    nc.tensor.matmul(op, h2[0][:, ds(rs * 128, 128)],
                                     w20_sb, start=True, stop=False)
                    nc.tensor.matmul(op, h2[1][:, ds(rs * 128, 128)],
                                     w21_sb, start=False, stop=False)
                    nc.tensor.matmul(op, ones_sb,
                                     b2p_sb, start=False, stop=True)
                    osb = osbp.tile([128, C], F32, tag="osb")
                    nc.vector.tensor_copy(osb, op[:, 0:C])
                    nc.scalar.dma_start(
                        out=out_d[ds(gp * 256 + rs * 128, 128), :], in_=osb
                    )

    # TRN2 allows at most 1 sync wait per instruction (2 on event semaphores).
    # Tile can emit more; split them the same way Bacc.compile() does.
    import bass_rust as _bass_rust
    _bass_rust.move_matmul_waits_to_ldweights(nc.m)
    _bass_rust.generate_event_semaphores(nc)
    return nc


def host_prep(output_ids, features, W1, b1, Wm, bm, W2, b2, rpc=RPC, tch=TCH):
    """Build per-core input maps. Heavy data (features) is passed as-is;
    the tiny one-hot/count/weight tensors are repacked for device layout."""
    S_ = tch * 128
    ids = np.asarray(output_ids)
    nrows = ids.shape[0]
    ncores = nrows // rpc
    feats = np.ascontiguousarray(np.asarray(features, dtype=np.float32))

    is_sep = ids == SEP_ID
    seg = np.cumsum(is_sep.astype(np.int64), axis=1)
    valid = ~is_sep
    oh = ((seg[:, :, None] == np.arange(NSEG)[None, None, :]) & valid[:, :, None])
    oh = oh.astype(np.float32)                        # [B, S, 8]
    counts = oh.sum(axis=1)                           # [B, 8]
    icnt_full = (1.0 / np.maximum(counts, 1.0)).astype(np.float32)

    # E4 [8, r4, g2, b2, i, j]: column (g2,b2,i,j) of 4-row block, row-chunk r4
    eye = np.eye(NSEG, dtype=np.float32)
    base = eye[:, :, None] - eye[:, None, :]          # [n, i, j]
    e4 = np.zeros((NSEG, 4, 2, 2, NSEG, NSEG), np.float32)
    for r4 in range(4):
        e4[:, r4, r4 // 2, r4 % 2, :, :] = base
    e4 = np.ascontiguousarray(e4.reshape(NSEG, 4 * 256))

    W1 = np.asarray(W1, np.float32)
    Wm = np.asarray(Wm, np.float32)
    W2 = np.asarray(W2, np.float32)
    b1 = np.asarray(b1, np.float32)
    bm = np.asarray(bm, np.float32)
    b2 = np.asarray(b2, np.float32)

    w1p = np.ascontiguousarray(
        W1.reshape(HC, 128, C).transpose(1, 0, 2).reshape(128, HC * C))
    wm0 = np.ascontiguousarray(Wm[:128])
    wm1 = np.ascontiguousarray(Wm[128:])
    w2pad = np.zeros((C, CPAD), np.float32)
    w2pad[:, :C] = W2
    w20 = np.ascontiguousarray(w2pad[:128])
    w21 = np.ascontiguousarray(w2pad[128:])
    b2pad = np.zeros((1, CPAD), np.float32)
    b2pad[0, :C] = b2
    b1c0 = np.ascontiguousarray(b1[:128, None])
    b1c1 = np.ascontiguousarray(b1[128:, None])
    bm0 = np.ascontiguousarray(bm[:128, None])
    bm1 = np.ascontiguousarray(bm[128:, None])

    shared = dict(w1p=w1p, wm0=wm0, wm1=wm1, w20=w20, w21=w21,
                  b1c0=b1c0, b1c1=b1c1, bm0=bm0, bm1=bm1, b2pad=b2pad, e4=e4,
                  ones=np.ones((1, 128), np.float32))

    in_maps = []
    for c in range(ncores):
        rows = slice(c * rpc, (c + 1) * rpc)
        ohT = np.ascontiguousarray(
            oh[rows].reshape(rpc, tch, 128, NSEG)
            .transpose(2, 0, 1, 3).reshape(128, rpc * tch * NSEG))
        icnt = np.ascontiguousarray(icnt_full[rows].T)
        in_maps.append(dict(
            features=np.ascontiguousarray(feats[rows]),
            ohT=ohT, icnt=icnt, **shared))
    return in_maps


def gather_output(core_outs, rpc=RPC):
    """[ngp*256, C] per core -> [8, 8, B, C]."""
    ncores = len(core_outs)
    ngp = rpc // 4
    full = np.empty((NSEG, NSEG, ncores * rpc, C), np.float32)
    for c, o in enumerate(core_outs):
        o = o.reshape(ngp, 2, 2, NSEG, NSEG, C)       # gp, g2, b2, i, j, c
        o = o.transpose(3, 4, 0, 1, 2, 5).reshape(NSEG, NSEG, rpc, C)
        full[:, :, c * rpc:(c + 1) * rpc, :] = o
    return full


_NC_CACHE = {}


def _get_program():
    key = (RPC, TCH, F32R_STAGE1, F32R_MLP)
    if key not in _NC_CACHE:
        _NC_CACHE[key] = build_program()
    return _NC_CACHE[key]


def run(inputs, trace=False, trace_cores=None):
    nc = _get_program()
    in_maps = host_prep(**inputs)
    res = run_bass_kernel_spmd(
        nc, in_maps, core_ids=list(range(NCORES)),
        trace=trace, trace_cores=trace_cores,
    )
    out = gather_output([r["out"] for r in res.results])
    return out, res


def kernel(**inputs):
    out, _ = run(inputs, trace=False)
    return out



# revision 2
# speedup vs baseline: 1.7797x; 1.7797x over previous
"""Trainium2 Bass kernel v2: segment-mean + pairwise-diff edge MLP, bf16 streaming.

Reference computation (per batch row b):
  seg = cumsum(ids == 3); valid = ids != 3
  means[n] = mean of features[s] over tokens with seg==n & valid (n < 8), 0-count -> sum/1
  diff[i,j] = means[i] - means[j]                          # [8,8,H]
  out[i,j]  = relu(relu(diff @ W1 + b1) @ Wm + bm) @ W2 + b2   # [8,8,150]

Distribution: data-parallel over batch B=128 across 8 NeuronCores (16 rows/core).

v2 changes vs v1:
  - features cast to bf16 on host: halves HBM traffic (the memory roofline).
  - per-core layout [ngp=4, 128, 24576]: group of 4 batch rows -> partition
    p=(r4,q), free=(t,h) with token = q*32+t. DMA is fully contiguous
    (48KB/partition per group), issued in 4 chunks of 1.57MB for pipelining.
  - stage 1 accumulates all 4 rows' segment sums into ONE [32, H] PSUM tile via
    a block-diagonal one-hot stationary [128, 32]; one scaled eviction per half.
  - pairwise diff: 1 matmul per h-chunk (stacked means x e4s [32,256]) instead
    of 4 zero-padded ones.
  - single batched output store at the end (no HOL blocking of feature DMAs).
"""

import sys

import numpy as np

if "/opt/trn_rl_repo" not in sys.path:
    sys.path.insert(0, "/opt/trn_rl_repo")

import ml_dtypes

import concourse.bass as bass
import concourse.mybir as mybir
from concourse.bass import ds
from concourse.bass_utils import run_bass_kernel_spmd
from concourse.tile import TileContext

B, S, H, C = 128, 1024, 768, 150
NSEG = 8
SEP_ID = 3
NCORES = 8
RPC = B // NCORES      # 16 rows per core
NGP = RPC // 4         # 4 groups of 4 rows
NT = 32                # token chunks per group (token = q*32 + t)
GPF = NT * H           # 24576 free elems per group
NCH = 4                # DMA chunks per group
CHF = GPF // NCH       # 6144 elems per chunk
HC = H // 128          # 6 hidden chunks
HSPLIT = ((0, 512), (512, 256))
CC = ((0, 128), (128, 22))  # c-dim (150) chunks
CPAD = 256

F32 = mybir.dt.float32
BF16 = mybir.dt.bfloat16
BF16NP = ml_dtypes.bfloat16


def build_program():
    nc = bass.Bass("TRN2", target_bir_lowering=False, debug=False)

    feats_d = nc.dram_tensor("features", [NGP, 128, GPF], BF16, kind="ExternalInput").ap()
    ohT4_d = nc.dram_tensor("ohT4", [128, NGP * NT * 32], BF16, kind="ExternalInput").ap()
    icnt_d = nc.dram_tensor("icnt", [32, NGP], F32, kind="ExternalInput").ap()
    e4s_d = nc.dram_tensor("e4s", [32, 256], BF16, kind="ExternalInput").ap()
    w1p_d = nc.dram_tensor("w1p", [128, HC * C], BF16, kind="ExternalInput").ap()
    wm0_d = nc.dram_tensor("wm0", [128, C], BF16, kind="ExternalInput").ap()
    wm1_d = nc.dram_tensor("wm1", [22, C], BF16, kind="ExternalInput").ap()
    w20_d = nc.dram_tensor("w20", [128, CPAD], BF16, kind="ExternalInput").ap()
    w21_d = nc.dram_tensor("w21", [22, CPAD], BF16, kind="ExternalInput").ap()
    b1c0_d = nc.dram_tensor("b1c0", [128, 1], F32, kind="ExternalInput").ap()
    b1c1_d = nc.dram_tensor("b1c1", [22, 1], F32, kind="ExternalInput").ap()
    bm0_d = nc.dram_tensor("bm0", [128, 1], F32, kind="ExternalInput").ap()
    bm1_d = nc.dram_tensor("bm1", [22, 1], F32, kind="ExternalInput").ap()
    b2p_d = nc.dram_tensor("b2pad", [1, CPAD], BF16, kind="ExternalInput").ap()
    ones_d = nc.dram_tensor("ones", [1, 128], BF16, kind="ExternalInput").ap()
    out_d = nc.dram_tensor("out", [NGP * 256, C], F32, kind="ExternalOutput").ap()

    RELU = mybir.ActivationFunctionType.Relu
    COPY = mybir.ActivationFunctionType.Copy

    with TileContext(nc) as tc:
        with (
            tc.tile_pool(name="const", bufs=1) as constp,
            tc.tile_pool(name="featp", bufs=3) as featp,
            tc.tile_pool(name="meansp", bufs=2) as meansp,
            tc.tile_pool(name="diffp", bufs=2) as diffp,
            tc.tile_pool(name="actp", bufs=2) as actp,
            tc.tile_pool(name="osbp", bufs=1) as osbp,
            tc.tile_pool(name="mpsum", bufs=1, space="PSUM") as mpsum,
            tc.tile_pool(name="dpsum", bufs=2, space="PSUM") as dpsum,
            tc.tile_pool(name="hpsum", bufs=2, space="PSUM") as hpsum,
            tc.tile_pool(name="opsum", bufs=2, space="PSUM") as opsum,
        ):
            ohT4_sb = constp.tile([128, NGP * NT * 32], BF16, tag="c_ohT4")
            nc.gpsimd.dma_start(out=ohT4_sb, in_=ohT4_d)
            icnt_sb = constp.tile([32, NGP], F32, tag="c_icnt")
            nc.gpsimd.dma_start(out=icnt_sb, in_=icnt_d)
            e4s_sb = constp.tile([32, 256], BF16, tag="c_e4s")
            nc.gpsimd.dma_start(out=e4s_sb, in_=e4s_d)
            w1_sb = constp.tile([128, HC * C], BF16, tag="c_w1")
            nc.gpsimd.dma_start(out=w1_sb, in_=w1p_d)
            wm0_sb = constp.tile([128, C], BF16, tag="c_wm0")
            nc.gpsimd.dma_start(out=wm0_sb, in_=wm0_d)
            wm1_sb = constp.tile([22, C], BF16, tag="c_wm1")
            nc.gpsimd.dma_start(out=wm1_sb, in_=wm1_d)
            w20_sb = constp.tile([128, CPAD], BF16, tag="c_w20")
            nc.gpsimd.dma_start(out=w20_sb, in_=w20_d)
            w21_sb = constp.tile([22, CPAD], BF16, tag="c_w21")
            nc.gpsimd.dma_start(out=w21_sb, in_=w21_d)
            b1_sb = []
            for ci, (coff, csz) in enumerate(CC):
                t = constp.tile([csz, 1], F32, tag=f"c_b1_{ci}")
                nc.gpsimd.dma_start(out=t, in_=(b1c0_d, b1c1_d)[ci])
                b1_sb.append(t)
            bm_sb = []
            for ci, (coff, csz) in enumerate(CC):
                t = constp.tile([csz, 1], F32, tag=f"c_bm_{ci}")
                nc.gpsimd.dma_start(out=t, in_=(bm0_d, bm1_d)[ci])
                bm_sb.append(t)
            b2p_sb = constp.tile([1, CPAD], BF16, tag="c_b2")
            nc.gpsimd.dma_start(out=b2p_sb, in_=b2p_d)
            ones_sb = constp.tile([1, 128], BF16, tag="c_ones")
            nc.gpsimd.dma_start(out=ones_sb, in_=ones_d)

            osb_full = osbp.tile([128, NGP * 2, C], F32, tag="osb")

            for gp in range(NGP):
                # ---- feature DMA: 4 chunks, fully contiguous, 2 HWDGE rings ----
                feat = featp.tile([128, GPF], BF16, tag="feat")
                for cq in range(NCH):
                    eng = (nc.sync, nc.scalar)[(gp * NCH + cq) % 2]
                    eng.dma_start(
                        out=feat[:, ds(cq * CHF, CHF)],
                        in_=feats_d[gp][:, ds(cq * CHF, CHF)],
                    )

                # ---- stage 1: segment sums for 4 rows into one [32, H] ----
                means = meansp.tile([32, H], BF16, tag="means")
                for hoff, hsz in HSPLIT:
                    mp = mpsum.tile([32, hsz], F32, tag=f"mp{hoff}")
                    for t in range(NT):
                        nc.tensor.matmul(
                            mp,
                            ohT4_sb[:, ds(gp * NT * 32 + t * 32, 32)],
                            feat[:, ds(t * H + hoff, hsz)],
                            start=(t == 0),
                            stop=(t == NT - 1),
                        )
                    nc.scalar.activation(
                        means[:, ds(hoff, hsz)], mp, COPY,
                        scale=icnt_sb[:, ds(gp, 1)],
                    )

                # ---- pairwise diff (fused transpose): diffT = means^T @ e4s ----
                diff = diffp.tile([128, HC, 256], BF16, tag="diff")
                for hc in range(HC):
                    dp = dpsum.tile([128, 256], F32, tag="dp")
                    nc.tensor.matmul(
                        dp, means[:, ds(hc * 128, 128)], e4s_sb,
                        start=True, stop=True,
                    )
                    nc.vector.tensor_copy(diff[:, hc, :], dp)

                # ---- mm1: h1T = relu(W1^T @ diffT + b1) ----
                h1 = []
                for ci, (coff, csz) in enumerate(CC):
                    hp = hpsum.tile([csz, 256], F32, tag="hp")
                    for hc in range(HC):
                        nc.tensor.matmul(
                            hp,
                            w1_sb[:, ds(hc * C + coff, csz)],
                            diff[:, hc, :],
                            start=(hc == 0),
                            stop=(hc == HC - 1),
                        )
                    hs = actp.tile([csz, 256], BF16, tag=f"h1s{ci}")
                    nc.scalar.activation(hs, hp, RELU, bias=b1_sb[ci])
                    h1.append(hs)

                # ---- mm2: h2T = relu(Wm^T @ h1T + bm) ----
                h2 = []
                for ci, (coff, csz) in enumerate(CC):
                    hp = hpsum.tile([csz, 256], F32, tag="hp")
                    nc.tensor.matmul(hp, wm0_sb[:, ds(coff, csz)],
                                     h1[0], start=True, stop=False)
                    nc.tensor.matmul(hp, wm1_sb[:, ds(coff, csz)],
                                     h1[1], start=False, stop=True)
                    hs = actp.tile([csz, 256], BF16, tag=f"h2s{ci}")
                    nc.scalar.activation(hs, hp, RELU, bias=bm_sb[ci])
                    h2.append(hs)

                # ---- mm3: out = h2 @ W2 + b2, natural [rows, c] layout ----
                for rs in range(2):
                    op = opsum.tile([128, CPAD], F32, tag="op")
                    nc.tensor.matmul(op, h2[0][:, ds(rs * 128, 128)],
                                     w20_sb, start=True, stop=False)
                    nc.tensor.matmul(op, h2[1][:, ds(rs * 128, 128)],
                                     w21_sb, start=False, stop=False)
                    nc.tensor.matmul(op, ones_sb,
                                     b2p_sb, start=False, stop=True)
                    nc.vector.tensor_copy(
                        osb_full[:, gp * 2 + rs, :], op[:, 0:C])

            # ---- single batched output store ----
            nc.sync.dma_start(
                out=out_d.rearrange("(g p) c -> p g c", p=128),
                in_=osb_full,
            )

    # TRN2 allows at most 1 sync wait per instruction (2 on event semaphores).
    # Tile can emit more; split them the same way Bacc.compile() does.
    import bass_rust as _bass_rust
    _bass_rust.move_matmul_waits_to_ldweights(nc.m)
    _bass_rust.generate_event_semaphores(nc)
    return nc


def host_prep(output_ids, features, W1, b1, Wm, bm, W2, b2):
    """Build per-core input maps. features cast to bf16 and viewed in the
    [ngp, 128, NT*H] interleaved layout; tiny tensors repacked for device."""
    ids = np.asarray(output_ids)
    nrows = ids.shape[0]
    ncores = nrows // RPC
    feats = np.asarray(features)
    if feats.dtype != BF16NP:
        feats = feats.astype(BF16NP)

    is_sep = ids == SEP_ID
    seg = np.cumsum(is_sep.astype(np.int64), axis=1)
    valid = ~is_sep
    oh = ((seg[:, :, None] == np.arange(NSEG)[None, None, :]) & valid[:, :, None])
    counts = oh.sum(axis=1)                           # [B, 8]
    icnt_full = (1.0 / np.maximum(counts, 1.0)).astype(np.float32)

    eye = np.eye(NSEG, dtype=np.float32)
    base = (eye[:, :, None] - eye[:, None, :]).reshape(NSEG, 64)  # [s, (i,j)]
    e4s = np.zeros((4, NSEG, 4, 64), np.float32)      # [r4, s, g2b2, (i,j)]
    for r4 in range(4):
        e4s[r4, :, r4, :] = base
    e4s = np.ascontiguousarray(e4s.reshape(32, 256)).astype(BF16NP)

    W1 = np.asarray(W1, np.float32)
    Wm = np.asarray(Wm, np.float32)
    W2 = np.asarray(W2, np.float32)
    b1 = np.asarray(b1, np.float32)
    bm = np.asarray(bm, np.float32)
    b2 = np.asarray(b2, np.float32)

    w1p = np.ascontiguousarray(
        W1.reshape(HC, 128, C).transpose(1, 0, 2).reshape(128, HC * C)
    ).astype(BF16NP)
    wm0 = np.ascontiguousarray(Wm[:128]).astype(BF16NP)
    wm1 = np.ascontiguousarray(Wm[128:]).astype(BF16NP)
    w2pad = np.zeros((C, CPAD), np.float32)
    w2pad[:, :C] = W2
    w20 = np.ascontiguousarray(w2pad[:128]).astype(BF16NP)
    w21 = np.ascontiguousarray(w2pad[128:]).astype(BF16NP)
    b2pad = np.zeros((1, CPAD), np.float32)
    b2pad[0, :C] = b2
    b2pad = b2pad.astype(BF16NP)
    b1c0 = np.ascontiguousarray(b1[:128, None])
    b1c1 = np.ascontiguousarray(b1[128:, None])
    bm0 = np.ascontiguousarray(bm[:128, None])
    bm1 = np.ascontiguousarray(bm[128:, None])

    shared = dict(e4s=e4s, w1p=w1p, wm0=wm0, wm1=wm1, w20=w20, w21=w21,
                  b1c0=b1c0, b1c1=b1c1, bm0=bm0, bm1=bm1, b2pad=b2pad,
                  ones=np.ones((1, 128), BF16NP))

    in_maps = []
    for c in range(ncores):
        rows = slice(c * RPC, (c + 1) * RPC)
        fc = np.ascontiguousarray(feats[rows]).reshape(NGP, 128, GPF)
        # one-hot, block-diagonal stationary: [r4, q, gp, t, r4', s]
        ohc = oh[rows].reshape(NGP, 4, 32, NT, NSEG)  # [gp, r4, q, t, s]
        ohT4 = np.zeros((4, 32, NGP, NT, 4, NSEG), np.float32)
        for r4 in range(4):
            ohT4[r4, :, :, :, r4, :] = ohc[:, r4].transpose(1, 0, 2, 3)
        ohT4 = np.ascontiguousarray(
            ohT4.reshape(128, NGP * NT * 32)).astype(BF16NP)
        icnt = np.ascontiguousarray(
            icnt_full[rows].reshape(NGP, 4, NSEG).transpose(1, 2, 0)
            .reshape(32, NGP))
        in_maps.append(dict(features=fc, ohT4=ohT4, icnt=icnt, **shared))
    return in_maps


def gather_output(core_outs):
    """[ngp*256, C] per core -> [8, 8, B, C]."""
    ncores = len(core_outs)
    full = np.empty((NSEG, NSEG, ncores * RPC, C), np.float32)
    for c, o in enumerate(core_outs):
        o = o.reshape(NGP, 2, 2, NSEG, NSEG, C)       # gp, g2, b2, i, j, c
        o = o.transpose(3, 4, 0, 1, 2, 5).reshape(NSEG, NSEG, RPC, C)
        full[:, :, c * RPC:(c + 1) * RPC, :] = o
    return full


_NC_CACHE = {}


def _get_program():
    if "nc" not in _NC_CACHE:
        _NC_CACHE["nc"] = build_program()
    return _NC_CACHE["nc"]


def run(inputs, trace=False, trace_cores=None):
    nc = _get_program()
    in_maps = host_prep(**inputs)
    res = run_bass_kernel_spmd(
        nc, in_maps, core_ids=list(range(NCORES)),
        trace=trace, trace_cores=trace_cores,
    )
    out = gather_output([r["out"] for r in res.results])
    return out, res


def kernel(**inputs):
    out, _ = run(inputs, trace=False)
    return out


# revision 3
# speedup vs baseline: 1.9649x; 1.1041x over previous
"""Trainium2 Bass kernel v3: segment-mean + pairwise-diff edge MLP, bf16 streaming.

Reference computation (per batch row b):
  seg = cumsum(ids == 3); valid = ids != 3
  means[n] = mean of features[s] over tokens with seg==n & valid (n < 8), 0-count -> sum/1
  diff[i,j] = means[i] - means[j]                          # [8,8,H]
  out[i,j]  = relu(relu(diff @ W1 + b1) @ Wm + bm) @ W2 + b2   # [8,8,150]

Distribution: data-parallel over batch B=128 across 8 NeuronCores (16 rows/core).

Layout (per core, 4 groups of 4 batch rows):
  features bf16 in [ngp, 128, 24576]: partition p=(r4,q), free=(t,h),
  token = q*32 + t -> DMA fully contiguous 48KB/partition per group,
  issued as 4x 1.57MB chunks on the sync HWDGE ring, preceded by that
  group's block-diagonal one-hot stationary slice (so stage-1 can start
  as soon as the first chunk lands). Stage 1 accumulates all 4 rows'
  segment sums into one [32, H] PSUM tile; scaled eviction -> bf16 means.
  Pairwise diff = one matmul per h-chunk vs a +-1 selection matrix (fuses
  transpose + subtraction). MLP weights live in one packed const DMA.
  Per-group output stores go on the otherwise-idle gpsimd SWDGE queue.
"""

import sys

import numpy as np

if "/opt/trn_rl_repo" not in sys.path:
    sys.path.insert(0, "/opt/trn_rl_repo")

import ml_dtypes

import concourse.bass as bass
import concourse.mybir as mybir
from concourse.bass import ds
from concourse.bass_utils import run_bass_kernel_spmd
from concourse.tile import TileContext

B, S, H, C = 128, 1024, 768, 150
NSEG = 8
SEP_ID = 3
NCORES = 8
RPC = B // NCORES      # 16 rows per core
NGP = RPC // 4         # 4 groups of 4 rows
NT = 32                # token chunks per group (token = q*32 + t)
GPF = NT * H           # 24576 free elems per group
NCH = 4                # DMA chunks per group
CHF = GPF // NCH       # 6144 elems per chunk
HC = H // 128          # 6 hidden chunks
HSPLIT = ((0, 512), (512, 256))
CC = ((0, 128), (128, 22))  # c-dim (150) chunks
CPAD = 256

F32 = mybir.dt.float32
BF16 = mybir.dt.bfloat16
BF16NP = ml_dtypes.bfloat16

# packed bf16 const block column offsets
PB_W1 = 0                      # [128, 900]
PB_WM0 = PB_W1 + HC * C        # [128, 150]
PB_WM1 = PB_WM0 + C            # [22, 150]
PB_W20 = PB_WM1 + C            # [128, 256]
PB_W21 = PB_W20 + CPAD         # [22, 256]
PB_B2 = PB_W21 + CPAD          # [1, 256]
PB_ONES = PB_B2 + CPAD         # [1, 128]
PB_E4S = PB_ONES + 128         # [32, 256]
PB_N = PB_E4S + 256            # 2352


def build_program():
    nc = bass.Bass("TRN2", target_bir_lowering=False, debug=False)

    feats_d = nc.dram_tensor("features", [NGP, 128, GPF], BF16, kind="ExternalInput").ap()
    ohT4_d = nc.dram_tensor("ohT4", [128, NGP * NT * 32], BF16, kind="ExternalInput").ap()
    constb_d = nc.dram_tensor("constb", [128, PB_N], BF16, kind="ExternalInput").ap()
    constf_d = nc.dram_tensor("constf", [128, 8], F32, kind="ExternalInput").ap()
    out_d = nc.dram_tensor("out", [NGP * 256, C], F32, kind="ExternalOutput").ap()

    RELU = mybir.ActivationFunctionType.Relu
    COPY = mybir.ActivationFunctionType.Copy

    with TileContext(nc) as tc:
        with (
            tc.tile_pool(name="const", bufs=1) as constp,
            tc.tile_pool(name="featp", bufs=3) as featp,
            tc.tile_pool(name="meansp", bufs=2) as meansp,
            tc.tile_pool(name="diffp", bufs=2) as diffp,
            tc.tile_pool(name="actp", bufs=2) as actp,
            tc.tile_pool(name="osbp", bufs=2) as osbp,
            tc.tile_pool(name="mpsum", bufs=1, space="PSUM") as mpsum,
            tc.tile_pool(name="dpsum", bufs=2, space="PSUM") as dpsum,
            tc.tile_pool(name="hpsum", bufs=2, space="PSUM") as hpsum,
            tc.tile_pool(name="opsum", bufs=2, space="PSUM") as opsum,
        ):
            constb = constp.tile([128, PB_N], BF16, tag="c_b")
            nc.gpsimd.dma_start(out=constb, in_=constb_d)
            constf = constp.tile([128, 8], F32, tag="c_f")
            nc.gpsimd.dma_start(out=constf, in_=constf_d)

            w1_sb = constb[:, ds(PB_W1, HC * C)]
            wm_sb = (constb[:, ds(PB_WM0, C)], constb[ds(0, 22), ds(PB_WM1, C)])
            w2_sb = (constb[:, ds(PB_W20, CPAD)], constb[ds(0, 22), ds(PB_W21, CPAD)])
            b2p_sb = constb[ds(0, 1), ds(PB_B2, CPAD)]
            ones_sb = constb[ds(0, 1), ds(PB_ONES, 128)]
            e4s_sb = constb[ds(0, 32), ds(PB_E4S, 256)]
            b1_sb = (constf[:, ds(0, 1)], constf[ds(0, 22), ds(1, 1)])
            bm_sb = (constf[:, ds(2, 1)], constf[ds(0, 22), ds(3, 1)])

            ohT4_sb = constp.tile([128, NGP * NT * 32], BF16, tag="c_ohT4")

            for gp in range(NGP):
                # ---- one-hot stationary slice, then 4 contiguous chunks ----
                nc.sync.dma_start(
                    out=ohT4_sb[:, ds(gp * NT * 32, NT * 32)],
                    in_=ohT4_d[:, ds(gp * NT * 32, NT * 32)],
                )
                feat = featp.tile([128, GPF], BF16, tag="feat")
                for cq in range(NCH):
                    nc.sync.dma_start(
                        out=feat[:, ds(cq * CHF, CHF)],
                        in_=feats_d[gp][:, ds(cq * CHF, CHF)],
                    )

                # ---- stage 1: segment sums for 4 rows into one [32, H] ----
                means = meansp.tile([32, H], BF16, tag="means")
                for hoff, hsz in HSPLIT:
                    mp = mpsum.tile([32, hsz], F32, tag=f"mp{hoff}")
                    for t in range(NT):
                        nc.tensor.matmul(
                            mp,
                            ohT4_sb[:, ds(gp * NT * 32 + t * 32, 32)],
                            feat[:, ds(t * H + hoff, hsz)],
                            start=(t == 0),
                            stop=(t == NT - 1),
                        )
                    nc.scalar.activation(
                        means[:, ds(hoff, hsz)], mp, COPY,
                        scale=constf[ds(0, 32), ds(4 + gp, 1)],
                    )

                # ---- pairwise diff (fused transpose): diffT = means^T @ e4s ----
                diff = diffp.tile([128, HC, 256], BF16, tag="diff")
                for hc in range(HC):
                    dp = dpsum.tile([128, 256], F32, tag="dp")
                    nc.tensor.matmul(
                        dp, means[:, ds(hc * 128, 128)], e4s_sb,
                        start=True, stop=True,
                    )
                    nc.vector.tensor_copy(diff[:, hc, :], dp)

                # ---- mm1: h1T = relu(W1^T @ diffT + b1) ----
                h1 = []
                for ci, (coff, csz) in enumerate(CC):
                    hp = hpsum.tile([csz, 256], F32, tag="hp")
                    for hc in range(HC):
                        nc.tensor.matmul(
                            hp,
                            w1_sb[:, ds(hc * C + coff, csz)],
                            diff[:, hc, :],
                            start=(hc == 0),
                            stop=(hc == HC - 1),
                        )
                    hs = actp.tile([csz, 256], BF16, tag=f"h1s{ci}")
                    nc.scalar.activation(hs, hp, RELU, bias=b1_sb[ci])
                    h1.append(hs)

                # ---- mm2: h2T = relu(Wm^T @ h1T + bm) ----
                h2 = []
                for ci, (coff, csz) in enumerate(CC):
                    hp = hpsum.tile([csz, 256], F32, tag="hp")
                    nc.tensor.matmul(hp, wm_sb[0][:, ds(coff, csz)],
                                     h1[0], start=True, stop=False)
                    nc.tensor.matmul(hp, wm_sb[1][:, ds(coff, csz)],
                                     h1[1], start=False, stop=True)
                    hs = actp.tile([csz, 256], BF16, tag=f"h2s{ci}")
                    nc.scalar.activation(hs, hp, RELU, bias=bm_sb[ci])
                    h2.append(hs)

                # ---- mm3: out = h2 @ W2 + b2, natural [rows, c] layout ----
                osb = osbp.tile([128, 2, C], F32, tag="osb")
                for rs in range(2):
                    op = opsum.tile([128, CPAD], F32, tag="op")
                    nc.tensor.matmul(op, h2[0][:, ds(rs * 128, 128)],
                                     w2_sb[0], start=True, stop=False)
                    nc.tensor.matmul(op, h2[1][:, ds(rs * 128, 128)],
                                     w2_sb[1], start=False, stop=False)
                    nc.tensor.matmul(op, ones_sb,
                                     b2p_sb, start=False, stop=True)
                    nc.vector.tensor_copy(osb[:, rs, :], op[:, 0:C])
                nc.gpsimd.dma_start(
                    out=out_d[ds(gp * 256, 256), :].rearrange(
                        "(g p) c -> p g c", p=128),
                    in_=osb,
                )

    # TRN2 allows at most 1 sync wait per instruction (2 on event semaphores).
    # Tile can emit more; split them the same way Bacc.compile() does.
    import bass_rust as _bass_rust
    _bass_rust.move_matmul_waits_to_ldweights(nc.m)
    _bass_rust.generate_event_semaphores(nc)
    return nc


def host_prep(output_ids, features, W1, b1, Wm, bm, W2, b2):
    """Build per-core input maps. features cast to bf16 and viewed in the
    [ngp, 128, NT*H] interleaved layout; tiny tensors repacked/packed."""
    ids = np.asarray(output_ids)
    nrows = ids.shape[0]
    ncores = nrows // RPC
    feats = np.asarray(features)
    if feats.dtype != BF16NP:
        feats = feats.astype(BF16NP)

    is_sep = ids == SEP_ID
    seg = np.cumsum(is_sep.astype(np.int64), axis=1)
    valid = ~is_sep
    oh = ((seg[:, :, None] == np.arange(NSEG)[None, None, :]) & valid[:, :, None])
    counts = oh.sum(axis=1)                           # [B, 8]
    icnt_full = (1.0 / np.maximum(counts, 1.0)).astype(np.float32)

    eye = np.eye(NSEG, dtype=np.float32)
    base = (eye[:, :, None] - eye[:, None, :]).reshape(NSEG, 64)  # [s, (i,j)]
    e4s = np.zeros((4, NSEG, 4, 64), np.float32)      # [r4, s, g2b2, (i,j)]
    for r4 in range(4):
        e4s[r4, :, r4, :] = base
    e4s = e4s.reshape(32, 256)

    W1 = np.asarray(W1, np.float32)
    Wm = np.asarray(Wm, np.float32)
    W2 = np.asarray(W2, np.float32)
    b1 = np.asarray(b1, np.float32)
    bm = np.asarray(bm, np.float32)
    b2 = np.asarray(b2, np.float32)

    constb = np.zeros((128, PB_N), np.float32)
    constb[:, PB_W1:PB_W1 + HC * C] = (
        W1.reshape(HC, 128, C).transpose(1, 0, 2).reshape(128, HC * C))
    constb[:, PB_WM0:PB_WM0 + C] = Wm[:128]
    constb[:22, PB_WM1:PB_WM1 + C] = Wm[128:]
    constb[:, PB_W20:PB_W20 + C] = W2[:128, :]
    constb[:22, PB_W21:PB_W21 + C] = W2[128:, :]
    constb[0, PB_B2:PB_B2 + C] = b2
    constb[0, PB_ONES:PB_ONES + 128] = 1.0
    constb[:32, PB_E4S:PB_E4S + 256] = e4s
    constb = constb.astype(BF16NP)

    constf_base = np.zeros((128, 8), np.float32)
    constf_base[:, 0] = b1[:128]
    constf_base[:22, 1] = b1[128:]
    constf_base[:, 2] = bm[:128]
    constf_base[:22, 3] = bm[128:]

    in_maps = []
    for c in range(ncores):
        rows = slice(c * RPC, (c + 1) * RPC)
        fc = np.ascontiguousarray(feats[rows]).reshape(NGP, 128, GPF)
        # one-hot, block-diagonal stationary: [r4, q, gp, t, r4', s]
        ohc = oh[rows].reshape(NGP, 4, 32, NT, NSEG)  # [gp, r4, q, t, s]
        ohT4 = np.zeros((4, 32, NGP, NT, 4, NSEG), np.float32)
        for r4 in range(4):
            ohT4[r4, :, :, :, r4, :] = ohc[:, r4].transpose(1, 0, 2, 3)
        ohT4 = np.ascontiguousarray(
            ohT4.reshape(128, NGP * NT * 32)).astype(BF16NP)
        constf = constf_base.copy()
        constf[:32, 4:8] = (
            icnt_full[rows].reshape(NGP, 4, NSEG).transpose(1, 2, 0)
            .reshape(32, NGP))
        in_maps.append(dict(features=fc, ohT4=ohT4, constb=constb,
                            constf=constf))
    return in_maps


def gather_output(core_outs):
    """[ngp*256, C] per core -> [8, 8, B, C]."""
    ncores = len(core_outs)
    full = np.empty((NSEG, NSEG, ncores * RPC, C), np.float32)
    for c, o in enumerate(core_outs):
        o = o.reshape(NGP, 2, 2, NSEG, NSEG, C)       # gp, g2, b2, i, j, c
        o = o.transpose(3, 4, 0, 1, 2, 5).reshape(NSEG, NSEG, RPC, C)
        full[:, :, c * RPC:(c + 1) * RPC, :] = o
    return full


_NC_CACHE = {}


def _get_program():
    if "nc" not in _NC_CACHE:
        _NC_CACHE["nc"] = build_program()
    return _NC_CACHE["nc"]


def run(inputs, trace=False, trace_cores=None):
    nc = _get_program()
    in_maps = host_prep(**inputs)
    res = run_bass_kernel_spmd(
        nc, in_maps, core_ids=list(range(NCORES)),
        trace=trace, trace_cores=trace_cores,
    )
    out = gather_output([r["out"] for r in res.results])
    return out, res


def kernel(**inputs):
    out, _ = run(inputs, trace=False)
    return out
